# revision 3
# baseline (speedup 1.0000x reference)
"""MoE MLP block (RMSNorm + top-4-of-32 router + 32-expert SwiGLU MLP +
weighted combine + residual) on 8 Trainium2 NeuronCores.

Strategy: expert-parallel. Core c owns experts 4c..4c+3 and receives only
their (layout-prepped) weights. The host computes routing metadata only
(top-k indices, capacity-bucket slots, combine weights — O(T*E) work);
every core then, on device: RMSNorms all T tokens, gathers its tokens by
indirect DMA, runs the expert MLP (biases folded into the matmuls via an
appended ones-row / bias-row), scales rows by the combine weights, and
gather-combines its contribution into a partial [T, H] output. The host
sums the 8 partials and adds the residual (pure data movement + a trivial
8-way add).

Weight layout prep (host, pure permutation):
  W1[e] columns are interleaved (even=glu, odd=linear). We permute columns
  into 6 chunk-pairs of [480 glu | 480 lin] so the device reads contiguous
  960-column blocks, and append b1 as a final row (ones-row trick adds the
  bias during PSUM accumulation). Same bias-row append for W2/b2.
"""

import functools
import sys

import numpy as np

sys.path.insert(0, "/opt/trn_rl_repo")

import ml_dtypes  # noqa: E402

import concourse.bass as bass  # noqa: E402
import concourse.tile as tile  # noqa: E402
from concourse import bacc, mybir  # noqa: E402
from concourse.bass_utils import run_bass_kernel_spmd  # noqa: E402
from concourse.masks import make_identity  # noqa: E402

BF16 = ml_dtypes.bfloat16

PROFILE = False      # set by test.py; harness leaves it False
LAST_EXEC_NS = None  # slowest-core HW exec time when PROFILE
LAST_TRACE = None    # perfetto trace path when PROFILE

T, H, I, E, K = 1024, 2880, 2880, 32, 4
LIMIT, ALPHA, EPS, CAP = 7.0, 1.702, 1e-5, 384
NCORES = 8
EPC = E // NCORES  # experts per core
CN = 480           # free-dim chunk width (PSUM bank holds 512 fp32)
NCH = I // CN      # 6 chunks over the glu/lin halves and over H
# Contraction stripes over 2880+1 rows (weights carry a bias row):
HS = [128] * 22 + [65]
HOFF = [128 * i for i in range(23)]

AF = mybir.ActivationFunctionType
ALU = mybir.AluOpType


# ---------------------------------------------------------------------------
# Device program
# ---------------------------------------------------------------------------
@functools.lru_cache(maxsize=4)
def _build_program(m_pad: int):
    dt = mybir.dt
    nc = bacc.Bacc(
        "TRN2", target_bir_lowering=False, debug=False, num_devices=NCORES
    )
    x_d = nc.dram_tensor("x", [T, H], dt.bfloat16, kind="ExternalInput").ap()
    scale_d = nc.dram_tensor(
        "norm_scale", [H], dt.float32, kind="ExternalInput"
    ).ap()
    w1_d = nc.dram_tensor(
        "w1p", [EPC, H + 1, 2 * I], dt.bfloat16, kind="ExternalInput"
    ).ap()
    w2_d = nc.dram_tensor(
        "w2p", [EPC, I + 1, H], dt.bfloat16, kind="ExternalInput"
    ).ap()
    tok_d = nc.dram_tensor(
        "disp_tok", [EPC, m_pad], dt.int32, kind="ExternalInput"
    ).ap()
    coef_d = nc.dram_tensor(
        "coef", [EPC, m_pad], dt.float32, kind="ExternalInput"
    ).ap()
    comb_d = nc.dram_tensor("comb", [T, K], dt.int32, kind="ExternalInput").ap()
    part_d = nc.dram_tensor(
        "partial", [T, H], dt.bfloat16, kind="ExternalOutput"
    ).ap()

    MT = m_pad // 128  # 128-row m-tiles per expert
    ZROW = EPC * m_pad  # index of the all-zero dummy row in o_buf

    with tile.TileContext(nc) as tc:
        with (
            tc.tile_pool(name="const", bufs=1) as const,
            tc.tile_pool(name="dram", bufs=1, space="DRAM") as dram,
        ):
            t_dram = dram.tile([T, H], dt.bfloat16)
            o_buf = dram.tile([ZROW + 1, H], dt.bfloat16)

            identity = const.tile([128, 128], dt.bfloat16)
            make_identity(nc, identity[:])
            eps_t = const.tile([128, 1], dt.float32)
            nc.vector.memset(eps_t[:], EPS)
            scale_bc = const.tile([128, H], dt.float32)
            nc.sync.dma_start(
                out=scale_bc[:],
                in_=bass.AP(
                    tensor=scale_d.tensor,
                    offset=scale_d.offset,
                    ap=[[0, 128]] + list(scale_d.ap),
                ),
            )
            zrow_t = const.tile([1, H], dt.bfloat16)
            nc.vector.memset(zrow_t[:], 0.0)
            nc.sync.dma_start(out=o_buf[ZROW : ZROW + 1, :], in_=zrow_t[:])

            # ---------------- RMSNorm: x -> t_dram --------------------------
            with tc.tile_pool(name="norm", bufs=3) as npool:
                for i in range(T // 128):
                    xt = npool.tile([128, H], dt.bfloat16, tag="xt")
                    nc.sync.dma_start(out=xt[:], in_=x_d[i * 128 : (i + 1) * 128, :])
                    dump = npool.tile([128, H], dt.bfloat16, tag="dump")
                    ssum = npool.tile([128, 1], dt.float32, tag="ssum")
                    nc.scalar.activation(
                        out=dump[:], in_=xt[:], func=AF.Square, accum_out=ssum[:]
                    )
                    # ssum <- 1/sqrt(mean + eps)
                    nc.scalar.activation(
                        out=ssum[:],
                        in_=ssum[:],
                        func=AF.Sqrt,
                        bias=eps_t[:],
                        scale=1.0 / H,
                    )
                    nc.vector.reciprocal(out=ssum[:], in_=ssum[:])
                    tf = npool.tile([128, H], dt.float32, tag="tf")
                    nc.vector.tensor_scalar_mul(
                        out=tf[:], in0=xt[:], scalar1=ssum[:]
                    )
                    tb = npool.tile([128, H], dt.bfloat16, tag="tb")
                    nc.vector.tensor_mul(out=tb[:], in0=tf[:], in1=scale_bc[:])
                    nc.sync.dma_start(
                        out=t_dram[i * 128 : (i + 1) * 128, :], in_=tb[:]
                    )

            # ---------------- Expert MLPs ----------------------------------
            with (
                tc.tile_pool(name="xe", bufs=2) as xep,
                tc.tile_pool(name="xeT", bufs=2) as xetp,
                tc.tile_pool(name="aT", bufs=2) as atp,
                tc.tile_pool(name="asb", bufs=2) as asbp,
                tc.tile_pool(name="osb", bufs=2) as osbp,
                tc.tile_pool(name="w1s", bufs=4) as w1p,
                tc.tile_pool(name="w2s", bufs=4) as w2p,
                tc.tile_pool(name="sw", bufs=3) as swp,
                tc.tile_pool(name="small", bufs=4) as smp,
                # 2 tags (A/B) per m-tile, 1 bank each; keep total <= 8 banks
                tc.tile_pool(
                    name="ps", bufs=(2 if MT <= 2 else 1), space="PSUM"
                ) as psp,
            ):
                for e in range(EPC):
                    # -- gather this expert's tokens and transpose to [H, M] --
                    xeT = [
                        xetp.tile([HS[h], m_pad], dt.bfloat16, tag=f"xeT{h}", name=f"xeT{e}_{h}")
                        for h in range(23)
                    ]
                    for m in range(MT):
                        idx = smp.tile([128, 1], dt.int32, tag="idx")
                        nc.sync.dma_start(
                            out=idx[:, 0:1],
                            in_=tok_d[e, m * 128 : (m + 1) * 128].rearrange(
                                "(a b) -> a b", b=1
                            ),
                        )
                        xe = xep.tile([128, H], dt.bfloat16, tag="xe")
                        nc.gpsimd.indirect_dma_start(
                            out=xe[:],
                            out_offset=None,
                            in_=t_dram[:],
                            in_offset=bass.IndirectOffsetOnAxis(
                                ap=idx[:, 0:1], axis=0
                            ),
                        )
                        for h in range(23):
                            hsz = HS[h] if h < 22 else 64
                            tp = psp.tile(
                                [hsz, 128],
                                dt.bfloat16,
                                space="PSUM",
                                tag=f"A{m % 2}",
                            )
                            nc.tensor.transpose(
                                out=tp[:],
                                in_=xe[:, HOFF[h] : HOFF[h] + hsz],
                                identity=identity[:],
                            )
                            nc.scalar.copy(
                                out=xeT[h][0:hsz, m * 128 : (m + 1) * 128],
                                in_=tp[:],
                            )
                    nc.vector.memset(xeT[22][64:65, :], 1.0)

                    # -- h = xe @ W1p (+b1), swiglu -> a ----------------------
                    a_sb = [
                        asbp.tile([128, I], dt.bfloat16, tag=f"a{m}", name=f"a{e}_{m}")
                        for m in range(MT)
                    ]
                    for n in range(NCH):
                        pg = [
                            psp.tile([128, CN], dt.float32, space="PSUM", tag=f"A{m}", name=f"pg{e}_{n}_{m}")
                            for m in range(MT)
                        ]
                        pl = [
                            psp.tile([128, CN], dt.float32, space="PSUM", tag=f"B{m}", name=f"pl{e}_{n}_{m}")
                            for m in range(MT)
                        ]
                        for h in range(23):
                            w1s = w1p.tile([HS[h], 2 * CN], dt.bfloat16, tag="w1s")
                            nc.sync.dma_start(
                                out=w1s[:],
                                in_=w1_d[
                                    e,
                                    HOFF[h] : HOFF[h] + HS[h],
                                    n * 2 * CN : (n + 1) * 2 * CN,
                                ],
                            )
                            for m in range(MT):
                                nc.tensor.matmul(
                                    out=pg[m][:],
                                    lhsT=xeT[h][:, m * 128 : (m + 1) * 128],
                                    rhs=w1s[:, 0:CN],
                                    start=(h == 0),
                                    stop=(h == 22),
                                )
                                nc.tensor.matmul(
                                    out=pl[m][:],
                                    lhsT=xeT[h][:, m * 128 : (m + 1) * 128],
                                    rhs=w1s[:, CN : 2 * CN],
                                    start=(h == 0),
                                    stop=(h == 22),
                                )
                        for m in range(MT):
                            xg = swp.tile([128, CN], dt.float32, tag="xg")
                            nc.vector.tensor_scalar_min(
                                out=xg[:], in0=pg[m][:], scalar1=LIMIT
                            )
                            sg = swp.tile([128, CN], dt.float32, tag="sg")
                            nc.scalar.activation(
                                out=sg[:], in_=xg[:], func=AF.Sigmoid, scale=ALPHA
                            )
                            nc.vector.tensor_mul(out=xg[:], in0=xg[:], in1=sg[:])
                            xl = swp.tile([128, CN], dt.float32, tag="xl")
                            nc.vector.tensor_scalar(
                                out=xl[:],
                                in0=pl[m][:],
                                scalar1=LIMIT,
                                scalar2=-LIMIT,
                                op0=ALU.min,
                                op1=ALU.max,
                            )
                            nc.vector.tensor_scalar_add(
                                out=xl[:], in0=xl[:], scalar1=1.0
                            )
                            nc.vector.tensor_mul(
                                out=a_sb[m][:, n * CN : (n + 1) * CN],
                                in0=xg[:],
                                in1=xl[:],
                            )

                    # -- transpose a -> aT [I+1, M] ---------------------------
                    aT = [
                        atp.tile([HS[h], m_pad], dt.bfloat16, tag=f"aT{h}", name=f"aT{e}_{h}")
                        for h in range(23)
                    ]
                    for m in range(MT):
                        for h in range(23):
                            hsz = HS[h] if h < 22 else 64
                            tp = psp.tile(
                                [hsz, 128],
                                dt.bfloat16,
                                space="PSUM",
                                tag=f"B{m % 2}",
                            )
                            nc.tensor.transpose(
                                out=tp[:],
                                in_=a_sb[m][:, HOFF[h] : HOFF[h] + hsz],
                                identity=identity[:],
                            )
                            nc.scalar.copy(
                                out=aT[h][0:hsz, m * 128 : (m + 1) * 128],
                                in_=tp[:],
                            )
                    nc.vector.memset(aT[22][64:65, :], 1.0)

                    # -- o = (a @ W2p (+b2)) * coef -> o_buf ------------------
                    o_sb = [
                        osbp.tile([128, H], dt.bfloat16, tag=f"o{m}", name=f"o{e}_{m}")
                        for m in range(MT)
                    ]
                    cf = smp.tile([128, MT], dt.float32, tag="cf")
                    nc.sync.dma_start(
                        out=cf[:, 0:MT],
                        in_=coef_d[e, :].rearrange("(m p) -> p m", p=128),
                    )
                    for n in range(NCH):
                        po = [
                            psp.tile([128, CN], dt.float32, space="PSUM", tag=f"A{m}", name=f"pg{e}_{n}_{m}")
                            for m in range(MT)
                        ]
                        for h in range(23):
                            w2s = w2p.tile([HS[h], CN], dt.bfloat16, tag="w2s")
                            nc.sync.dma_start(
                                out=w2s[:],
                                in_=w2_d[
                                    e,
                                    HOFF[h] : HOFF[h] + HS[h],
                                    n * CN : (n + 1) * CN,
                                ],
                            )
                            for m in range(MT):
                                nc.tensor.matmul(
                                    out=po[m][:],
                                    lhsT=aT[h][:, m * 128 : (m + 1) * 128],
                                    rhs=w2s[:],
                                    start=(h == 0),
                                    stop=(h == 22),
                                )
                        for m in range(MT):
                            nc.vector.tensor_scalar_mul(
                                out=o_sb[m][:, n * CN : (n + 1) * CN],
                                in0=po[m][:],
                                scalar1=cf[:, m : m + 1],
                            )
                    for m in range(MT):
                        r0 = e * m_pad + m * 128
                        nc.sync.dma_start(
                            out=o_buf[r0 : r0 + 128, :], in_=o_sb[m][:]
                        )

            # ---------------- combine: partial[t] = sum_k w*o ----------------
            with tc.tile_pool(name="comb", bufs=2) as cbp:
                for i in range(T // 128):
                    ci = cbp.tile([128, K], dt.int32, tag="ci")
                    nc.sync.dma_start(
                        out=ci[:], in_=comb_d[i * 128 : (i + 1) * 128, :]
                    )
                    gk = [
                        cbp.tile([128, H], dt.bfloat16, tag=f"g{k}", name=f"g{i}_{k}")
                        for k in range(K)
                    ]
                    for k in range(K):
                        nc.gpsimd.indirect_dma_start(
                            out=gk[k][:],
                            out_offset=None,
                            in_=o_buf[:],
                            in_offset=bass.IndirectOffsetOnAxis(
                                ap=ci[:, k : k + 1], axis=0
                            ),
                        )
                    s01 = cbp.tile([128, H], dt.float32, tag="s01")
                    nc.vector.tensor_add(out=s01[:], in0=gk[0][:], in1=gk[1][:])
                    s23 = cbp.tile([128, H], dt.float32, tag="s23")
                    nc.vector.tensor_add(out=s23[:], in0=gk[2][:], in1=gk[3][:])
                    pt = cbp.tile([128, H], dt.bfloat16, tag="pt")
                    nc.vector.tensor_add(out=pt[:], in0=s01[:], in1=s23[:])
                    nc.sync.dma_start(
                        out=part_d[i * 128 : (i + 1) * 128, :], in_=pt[:]
                    )

    nc.compile()
    return nc


# ---------------------------------------------------------------------------
# Host-side routing (mirrors reference semantics; O(T*E) work only)
# ---------------------------------------------------------------------------
def _route(x, norm_scale, gate_w, gate_b):
    xf = np.asarray(x, dtype=np.float32)
    ms = np.mean(xf * xf, axis=-1, keepdims=True)
    t32 = xf / np.sqrt(ms + EPS)
    t32 = t32 * np.asarray(norm_scale, dtype=np.float32)
    tb = t32.astype(BF16).astype(np.float32)
    g = (tb @ np.asarray(gate_w, dtype=np.float32)).astype(BF16).astype(np.float32)
    g = (g + np.asarray(gate_b, dtype=np.float32)).astype(BF16).astype(np.float32)
    # top-k with lowest-index tie-break (matches jax.lax.top_k)
    top_idx = np.argsort(-g, axis=-1, kind="stable")[:, :K].astype(np.int32)
    top_vals = np.take_along_axis(g, top_idx, axis=-1)
    ex = np.exp(top_vals - top_vals.max(axis=-1, keepdims=True))
    top_w = (ex / ex.sum(axis=-1, keepdims=True)).astype(BF16).astype(np.float32)

    N = T * K
    e_flat = top_idx.reshape(N)
    w_flat = top_w.reshape(N)
    tok_flat = np.repeat(np.arange(T, dtype=np.int32), K)
    order = np.argsort(e_flat, kind="stable")
    se, sw, stok = e_flat[order], w_flat[order], tok_flat[order]
    counts = np.bincount(se, minlength=E)
    starts = np.cumsum(counts) - counts
    pos = np.arange(N, dtype=np.int64) - starts[se]
    valid = pos < CAP
    return order, se, sw, stok, pos, valid, counts


def kernel(x, norm_scale, gate_w, gate_b, W1, b1, W2, b2):
    x = np.asarray(x)
    order, se, sw, stok, pos, valid, counts = _route(
        x, norm_scale, gate_w, gate_b
    )

    m_pad = int(min(CAP, max(128, ((counts.max() + 127) // 128) * 128)))
    MT = m_pad // 128
    ZROW = EPC * m_pad

    # fixed-capacity dispatch buffers, truncated to m_pad
    disp_tok = np.zeros((E, m_pad), np.int32)
    disp_cf = np.zeros((E, m_pad), np.float32)
    ok = valid & (pos < m_pad)
    disp_tok[se[ok], pos[ok]] = stok[ok]
    disp_cf[se[ok], pos[ok]] = sw[ok]

    # combine gather indices: for (t, k) -> local o_buf row on the owning core
    comb = np.full((NCORES, T, K), ZROW, np.int32)
    core_of = se // EPC
    loc_e = se % EPC
    k_of = (order % K).astype(np.int32)  # original k-slot of each sorted pair
    comb[core_of[ok], stok[ok], k_of[ok]] = (
        loc_e[ok] * m_pad + pos[ok]
    ).astype(np.int32)

    # per-core weight prep: de-interleave W1 columns into 6x[480 glu|480 lin]
    # chunk-pairs, append bias rows
    j = np.arange(CN)
    col_perm = np.concatenate(
        [np.concatenate([2 * (n * CN + j), 2 * (n * CN + j) + 1]) for n in range(NCH)]
    )
    W1 = np.asarray(W1)
    b1 = np.asarray(b1)
    W2 = np.asarray(W2)
    b2 = np.asarray(b2)

    nc = _build_program(m_pad)
    in_maps = []
    xb = np.ascontiguousarray(x.astype(BF16))
    sc = np.ascontiguousarray(np.asarray(norm_scale, dtype=np.float32))
    for c in range(NCORES):
        es = slice(c * EPC, (c + 1) * EPC)
        w1p = np.empty((EPC, H + 1, 2 * I), BF16)
        w1p[:, :H, :] = W1[es][:, :, col_perm]
        w1p[:, H, :] = b1[es][:, col_perm]
        w2p = np.empty((EPC, I + 1, H), BF16)
        w2p[:, :I, :] = W2[es]
        w2p[:, I, :] = b2[es]
        in_maps.append(
            {
                "x": xb,
                "norm_scale": sc,
                "w1p": w1p,
                "w2p": w2p,
                "disp_tok": disp_tok[es],
                "coef": disp_cf[es],
                "comb": np.ascontiguousarray(comb[c]),
            }
        )

    global LAST_EXEC_NS, LAST_TRACE
    res = run_bass_kernel_spmd(
        nc, in_maps, core_ids=list(range(NCORES)), trace=PROFILE
    )
    if PROFILE:
        LAST_EXEC_NS = res.exec_time_ns
        LAST_TRACE = (
            res.instructions_and_trace[1] if res.instructions_and_trace else None
        )
    acc = np.asarray(x, dtype=np.float32).copy()
    for c in range(NCORES):
        acc += res.results[c]["partial"].astype(np.float32)
    return acc.astype(BF16)



# revision 7
# speedup vs baseline: 128.5251x; 128.5251x over previous
"""MoE MLP block (RMSNorm + top-4-of-32 router + 32-expert SwiGLU MLP +
weighted combine + residual) on 8 Trainium2 NeuronCores.

Expert-parallel with count-aware slot packing: the host routes tokens
(reusing the RMSNorm it computes for the router logits as the device
token table), bin-packs the 32 experts into 8 cores x a shared slot
pattern (e.g. [2,2,1,1] m-tiles of 128 rows) so padded matmul work
tracks the real expert load, and ships layout-prepped weights.

Per core, per expert slot, the device:
  - dma_gather(transpose=True): gathers the slot's token rows from the
    normed token table AND transposes them into [128, stripe, M] lhsT
    layout in one SWDGE DMA (a ones-column at position 2880 folds b1).
  - W1 matmuls (full 5.65MB weight chunks; double-buffered), swiglu,
    PE-transpose a -> aT, W2 matmuls (+b2 via ones row), combine-weight
    row scaling.
  - dma_scatter_add: accumulates the weighted rows straight into the
    per-core partial output in DRAM (padding rows carry zeros into a
    dummy row; the runtime pre-zeros ExternalOutput buffers).

Chunk-level software pipelining keeps the tensor engine in long bursts
(p-state) while the weight stream saturates the DMA engines: slot s-1's
W2 chunk matmuls interleave between slot s's W1 chunk loads; gathers
prefetch one slot ahead; the opening W1 load issues from the SWDGE
queue so the first gathers win the DMA-pool FIFO. The host sums the 8
partials and adds the residual.
"""

import functools
import sys

import numpy as np

sys.path.insert(0, "/opt/trn_rl_repo")

import ml_dtypes  # noqa: E402

import concourse.bass as bass  # noqa: E402
import concourse.tile as tile  # noqa: E402
from concourse import bacc, mybir  # noqa: E402
from concourse.bass_utils import run_bass_kernel_spmd  # noqa: E402
from concourse.masks import make_identity  # noqa: E402

BF16 = ml_dtypes.bfloat16

PROFILE = False      # set by test.py; harness leaves it False
LAST_EXEC_NS = None  # slowest-core HW exec time when PROFILE
LAST_TRACE = None    # perfetto trace path when PROFILE
LAST_PAT = None      # slot pattern of the last kernel() call

T, H, I, E, K = 1024, 2880, 2880, 32, 4
LIMIT, ALPHA, EPS, CAP = 7.0, 1.702, 1e-5, 384
NCORES = 8
NSLOT = E // NCORES   # expert slots per core
HP = 2944             # padded row length = 23 * 128 (H + bias + zeros)
NS = 23               # contraction stripes of 128 over HP
CN = 480              # free-dim chunk width (PSUM bank holds 512 fp32)
NCH = I // CN         # 6 chunks over the glu/lin halves and over H

AF = mybir.ActivationFunctionType
ALU = mybir.AluOpType


# ---------------------------------------------------------------------------
# Device program
# ---------------------------------------------------------------------------
@functools.lru_cache(maxsize=4)
def _build_program(pat: tuple):
    """pat: per-slot m-tile counts, e.g. (2, 2, 1, 1)."""
    dt = mybir.dt
    nc = bacc.Bacc(
        "TRN2", target_bir_lowering=False, debug=False, num_devices=NCORES
    )
    MTOT = sum(pat)  # total m-tiles per core
    t_d = nc.dram_tensor("tnorm", [T, HP], dt.bfloat16, kind="ExternalInput").ap()
    w1_d = nc.dram_tensor(
        "w1p", [NSLOT, HP, 2 * I], dt.bfloat16, kind="ExternalInput"
    ).ap()
    w2_d = nc.dram_tensor(
        "w2p", [NSLOT, HP, H], dt.bfloat16, kind="ExternalInput"
    ).ap()
    gidx_d = nc.dram_tensor(
        "gidx", [MTOT, 128, 8], dt.int16, kind="ExternalInput"
    ).ap()
    sidx_d = nc.dram_tensor(
        "sidx", [MTOT, 128, 8], dt.int16, kind="ExternalInput"
    ).ap()
    coef_d = nc.dram_tensor(
        "coef", [MTOT, 128], dt.float32, kind="ExternalInput"
    ).ap()
    # partial is pre-zeroed by the runtime (ExternalOutput semantics)
    part_d = nc.dram_tensor(
        "partial", [T + 128, HP], dt.bfloat16, kind="ExternalOutput"
    ).ap()

    mt0 = [0]
    for s in range(NSLOT):
        mt0.append(mt0[-1] + pat[s])

    with tile.TileContext(nc) as tc:
        with (
            tc.tile_pool(name="const", bufs=1) as const,
            tc.tile_pool(name="xeT", bufs=2) as xetp,
            tc.tile_pool(name="aT", bufs=1) as atp,
            tc.tile_pool(name="asb", bufs=1) as asbp,
            tc.tile_pool(name="osb", bufs=1) as osbp,
            tc.tile_pool(name="w1s", bufs=2) as w1p,
            tc.tile_pool(name="w2s", bufs=2) as w2p,
            tc.tile_pool(name="sw", bufs=2) as swp,
            tc.tile_pool(name="small", bufs=4) as smp,
            tc.tile_pool(name="ps1", bufs=1, space="PSUM") as psp,
            tc.tile_pool(name="ps2", bufs=2, space="PSUM") as pst,
        ):
            identity = const.tile([128, 128], dt.bfloat16)
            make_identity(nc, identity[:])

            # live per-slot state
            xeT = {}   # slot -> [m] tiles [128, NS*128]
            a_sb = {}  # slot -> [m] tiles [128, I]
            aT = {}    # slot -> [m] tiles [128, NS*128]
            o_sb = {}  # slot -> [m] tiles [128, HP]
            cf = {}    # slot -> tile [128, MT]

            def gather(s):
                MT = pat[s]
                xeT[s] = [
                    xetp.tile([128, NS * 128], dt.bfloat16,
                              tag=f"xeT{m}", name=f"xeT{s}_{m}")
                    for m in range(MT)
                ]
                for m in range(MT):
                    gi = smp.tile([128, 8], dt.int16, tag=f"gi{m}")
                    nc.sync.dma_start(out=gi[:], in_=gidx_d[mt0[s] + m, :, :])
                    nc.gpsimd.dma_gather(
                        out_ap=xeT[s][m][:].rearrange("p (s j) -> p s j", s=NS),
                        in_ap=t_d[:, :],
                        idxs_ap=gi[:],
                        num_idxs=128,
                        num_idxs_reg=128,
                        elem_size=HP,
                        transpose=True,
                    )

            def w1_load(s, n, eng=None):
                w1s = w1p.tile([128, NS * 2 * CN], dt.bfloat16, tag="w1s")
                (eng or nc.sync).dma_start(
                    out=w1s[:].rearrange("p (s c) -> p s c", s=NS),
                    in_=w1_d[
                        s, :, n * 2 * CN : (n + 1) * 2 * CN
                    ].rearrange("(s p) c -> p s c", p=128),
                )
                return w1s

            def w1_matmuls(s, n, w1s):
                MT = pat[s]
                pg = [
                    psp.tile([128, CN], dt.float32, space="PSUM",
                             tag=f"A{m}", name=f"pg{s}_{n}_{m}")
                    for m in range(MT)
                ]
                pl = [
                    psp.tile([128, CN], dt.float32, space="PSUM",
                             tag=f"B{m}", name=f"pl{s}_{n}_{m}")
                    for m in range(MT)
                ]
                for st in range(NS):
                    c0 = st * 2 * CN
                    for m in range(MT):
                        nc.tensor.matmul(
                            out=pg[m][:],
                            lhsT=xeT[s][m][:, st * 128 : (st + 1) * 128],
                            rhs=w1s[:, c0 : c0 + CN],
                            start=(st == 0),
                            stop=(st == NS - 1),
                        )
                        nc.tensor.matmul(
                            out=pl[m][:],
                            lhsT=xeT[s][m][:, st * 128 : (st + 1) * 128],
                            rhs=w1s[:, c0 + CN : c0 + 2 * CN],
                            start=(st == 0),
                            stop=(st == NS - 1),
                        )
                return pg, pl

            def swiglu(s, n, pg, pl):
                MT = pat[s]
                if n == 0:
                    a_sb[s] = [
                        asbp.tile([128, I], dt.bfloat16, tag=f"a{m}",
                                  name=f"a{s}_{m}")
                        for m in range(MT)
                    ]
                for m in range(MT):
                    xg = swp.tile([128, CN], dt.float32, tag="xg")
                    nc.vector.tensor_scalar_min(
                        out=xg[:], in0=pg[m][:], scalar1=LIMIT
                    )
                    sg = swp.tile([128, CN], dt.float32, tag="sg")
                    nc.scalar.activation(
                        out=sg[:], in_=xg[:], func=AF.Sigmoid, scale=ALPHA
                    )
                    nc.vector.tensor_mul(out=xg[:], in0=xg[:], in1=sg[:])
                    xl = swp.tile([128, CN], dt.float32, tag="xl")
                    nc.vector.tensor_scalar(
                        out=xl[:],
                        in0=pl[m][:],
                        scalar1=LIMIT,
                        scalar2=-LIMIT,
                        op0=ALU.min,
                        op1=ALU.max,
                    )
                    nc.vector.tensor_scalar_add(out=xl[:], in0=xl[:], scalar1=1.0)
                    nc.vector.tensor_mul(
                        out=a_sb[s][m][:, n * CN : (n + 1) * CN],
                        in0=xg[:],
                        in1=xl[:],
                    )

            def transposes(s):
                MT = pat[s]
                aT[s] = [
                    atp.tile([128, NS * 128], dt.bfloat16, tag=f"aT{m}",
                             name=f"aT{s}_{m}")
                    for m in range(MT)
                ]
                for m in range(MT):
                    for st in range(NS):
                        hsz = 128 if st < 22 else 64
                        tp = pst.tile([hsz, 128], dt.bfloat16, space="PSUM",
                                      tag="T")
                        nc.tensor.transpose(
                            out=tp[:],
                            in_=a_sb[s][m][:, st * 128 : st * 128 + hsz],
                            identity=identity[:],
                        )
                        nc.scalar.copy(
                            out=aT[s][m][0:hsz, st * 128 : (st + 1) * 128],
                            in_=tp[:],
                        )
                        if st == 22:
                            nc.vector.memset(
                                aT[s][m][64:128, st * 128 : (st + 1) * 128], 0.0
                            )
                            nc.vector.memset(
                                aT[s][m][64:65, st * 128 : (st + 1) * 128], 1.0
                            )
                # combine coefficients for this slot
                cf[s] = smp.tile([128, MT], dt.float32, tag="cf",
                                 name=f"cf{s}")
                for m in range(MT):
                    nc.sync.dma_start(
                        out=cf[s][:, m : m + 1],
                        in_=coef_d[mt0[s] + m, :].rearrange("(a b) -> a b", b=1),
                    )

            def w2_load(s, n):
                w2s = w2p.tile([128, NS * CN], dt.bfloat16, tag="w2s")
                nc.sync.dma_start(
                    out=w2s[:].rearrange("p (s c) -> p s c", s=NS),
                    in_=w2_d[s, :, n * CN : (n + 1) * CN].rearrange(
                        "(s p) c -> p s c", p=128
                    ),
                )
                return w2s

            def w2_matmuls(s, n, w2s):
                MT = pat[s]
                if n == 0:
                    o_sb[s] = [
                        osbp.tile([128, HP], dt.bfloat16, tag=f"o{m}",
                                  name=f"o{s}_{m}")
                        for m in range(MT)
                    ]
                po = [
                    psp.tile([128, CN], dt.float32, space="PSUM",
                             tag=f"C{m}", name=f"po{s}_{n}_{m}")
                    for m in range(MT)
                ]
                for st in range(NS):
                    c0 = st * CN
                    for m in range(MT):
                        nc.tensor.matmul(
                            out=po[m][:],
                            lhsT=aT[s][m][:, st * 128 : (st + 1) * 128],
                            rhs=w2s[:, c0 : c0 + CN],
                            start=(st == 0),
                            stop=(st == NS - 1),
                        )
                for m in range(MT):
                    nc.vector.tensor_scalar_mul(
                        out=o_sb[s][m][:, n * CN : (n + 1) * CN],
                        in0=po[m][:],
                        scalar1=cf[s][:, m : m + 1],
                    )

            def scatter(s):
                MT = pat[s]
                for m in range(MT):
                    nc.vector.memset(o_sb[s][m][:, H:HP], 0.0)
                    si = smp.tile([128, 8], dt.int16, tag=f"si{m}")
                    nc.sync.dma_start(out=si[:], in_=sidx_d[mt0[s] + m, :, :])
                    nc.gpsimd.dma_scatter_add(
                        part_d[:, :],
                        o_sb[s][m][:].rearrange("p (a c) -> p a c", a=1),
                        si[:],
                        128,
                        128,
                        HP,
                    )

            # ---- pipeline ----
            # First two slots' gathers go first; the opening W1 load is
            # issued from the Pool (SWDGE) queue so its descriptors are
            # generated strictly after the gathers' — the gathers' small
            # transfers win the DMA-pool FIFO and the PE cold start shrinks.
            gather(0)
            gather(1)
            w1s = w1_load(0, 0, eng=nc.gpsimd)
            for n in range(NCH):
                pg, pl = w1_matmuls(0, n, w1s)
                if n + 1 < NCH:
                    w1s = w1_load(0, n + 1)
                swiglu(0, n, pg, pl)
            for s in range(1, NSLOT):
                transposes(s - 1)
                if s + 1 < NSLOT:
                    gather(s + 1)
                for n in range(NCH):
                    w1s = w1_load(s, n)
                    pg, pl = w1_matmuls(s, n, w1s)
                    w2s = w2_load(s - 1, n)
                    w2_matmuls(s - 1, n, w2s)
                    swiglu(s, n, pg, pl)
                scatter(s - 1)
            transposes(NSLOT - 1)
            for n in range(NCH):
                w2s = w2_load(NSLOT - 1, n)
                w2_matmuls(NSLOT - 1, n, w2s)
            scatter(NSLOT - 1)

    nc.compile()
    return nc


# ---------------------------------------------------------------------------
# Host-side routing (mirrors reference semantics; O(T*E) work only)
# ---------------------------------------------------------------------------
def _route(x, norm_scale, gate_w, gate_b):
    xf = np.asarray(x, dtype=np.float32)
    ms = np.mean(xf * xf, axis=-1, keepdims=True)
    t32 = xf / np.sqrt(ms + EPS)
    t32 = t32 * np.asarray(norm_scale, dtype=np.float32)
    tb = t32.astype(BF16)
    tbf = tb.astype(np.float32)
    g = (tbf @ np.asarray(gate_w, dtype=np.float32)).astype(BF16).astype(np.float32)
    g = (g + np.asarray(gate_b, dtype=np.float32)).astype(BF16).astype(np.float32)
    # top-k with lowest-index tie-break (matches jax.lax.top_k)
    top_idx = np.argsort(-g, axis=-1, kind="stable")[:, :K].astype(np.int32)
    top_vals = np.take_along_axis(g, top_idx, axis=-1)
    ex = np.exp(top_vals - top_vals.max(axis=-1, keepdims=True))
    top_w = (ex / ex.sum(axis=-1, keepdims=True)).astype(BF16).astype(np.float32)

    N = T * K
    e_flat = top_idx.reshape(N)
    w_flat = top_w.reshape(N)
    tok_flat = np.repeat(np.arange(T, dtype=np.int32), K)
    order = np.argsort(e_flat, kind="stable")
    se, sw, stok = e_flat[order], w_flat[order], tok_flat[order]
    counts = np.bincount(se, minlength=E)
    starts = np.cumsum(counts) - counts
    pos = np.arange(N, dtype=np.int64) - starts[se]
    valid = pos < CAP
    return tb, se, sw, stok, pos, valid, counts


def _wrap16(idx):
    """[M] int -> [128, M//16] int16: idx i at [i % 16, i // 16], replicated
    into each of the 8 q7-core 16-partition groups."""
    m = idx.shape[0]
    w = np.zeros((128, m // 16), np.int16)
    w[:16, :] = idx.astype(np.int16).reshape(m // 16, 16).T
    w[:] = np.tile(w[:16], (8, 1))
    return w


def kernel(x, norm_scale, gate_w, gate_b, W1, b1, W2, b2):
    x = np.asarray(x)
    tb, se, sw, stok, pos, valid, counts = _route(x, norm_scale, gate_w, gate_b)

    # ---- bin-pack experts into 8 cores x slot pattern -------------------
    tiles = np.maximum(1, np.ceil(np.minimum(counts, CAP) / 128).astype(int))
    order_e = np.argsort(-tiles, kind="stable")  # big experts first
    core_exp = [[] for _ in range(NCORES)]
    core_load = np.zeros(NCORES, int)
    for e in order_e:
        c = np.argmin(
            core_load + (np.array([len(ce) for ce in core_exp]) >= NSLOT) * 1000
        )
        core_exp[int(c)].append(int(e))
        core_load[int(c)] += tiles[e]
    pat = np.zeros(NSLOT, int)
    for c in range(NCORES):
        ts = sorted([tiles[e] for e in core_exp[c]], reverse=True)
        for s in range(NSLOT):
            pat[s] = max(pat[s], ts[s])
        core_exp[c] = sorted(core_exp[c], key=lambda e: -tiles[e])
    pat = tuple(int(v) for v in pat)
    MTOT = sum(pat)

    # ---- dispatch metadata ---------------------------------------------
    gidx = np.zeros((NCORES, MTOT, 128), np.int32)
    sidx = np.full((NCORES, MTOT, 128), T, np.int32)  # dummy row = T
    coef = np.zeros((NCORES, MTOT, 128), np.float32)
    ok = valid
    for c in range(NCORES):
        m0 = 0
        for s_i, e in enumerate(core_exp[c]):
            sel = ok & (se == e) & (pos < pat[s_i] * 128)
            tok_e = stok[sel]
            w_e = sw[sel]
            ncnt = tok_e.shape[0]
            gidx[c, m0 : m0 + pat[s_i]].reshape(-1)[:ncnt] = tok_e
            sidx[c, m0 : m0 + pat[s_i]].reshape(-1)[:ncnt] = tok_e
            coef[c, m0 : m0 + pat[s_i]].reshape(-1)[:ncnt] = w_e
            m0 += pat[s_i]

    # ---- token table: normed tokens + ones column + zero pad ------------
    t_pad = np.zeros((T, HP), BF16)
    t_pad[:, :H] = tb
    t_pad[:, H] = BF16(1.0)

    # ---- per-core weight prep ------------------------------------------
    j = np.arange(CN)
    col_perm = np.concatenate(
        [np.concatenate([2 * (n * CN + j), 2 * (n * CN + j) + 1]) for n in range(NCH)]
    )
    W1 = np.asarray(W1)
    b1 = np.asarray(b1)
    W2 = np.asarray(W2)
    b2 = np.asarray(b2)

    global LAST_PAT
    LAST_PAT = pat
    nc = _build_program(pat)
    in_maps = []
    for c in range(NCORES):
        es = core_exp[c]
        w1p = np.zeros((NSLOT, HP, 2 * I), BF16)
        w1p[:, :H, :] = W1[es][:, :, col_perm]
        w1p[:, H, :] = b1[es][:, col_perm]
        w2p = np.zeros((NSLOT, HP, H), BF16)
        w2p[:, :H, :] = W2[es]
        w2p[:, H, :] = b2[es]
        in_maps.append(
            {
                "tnorm": t_pad,
                "w1p": w1p,
                "w2p": w2p,
                "gidx": np.stack([_wrap16(gidx[c, m]) for m in range(MTOT)]),
                "sidx": np.stack([_wrap16(sidx[c, m]) for m in range(MTOT)]),
                "coef": coef[c],
            }
        )

    global LAST_EXEC_NS, LAST_TRACE
    res = run_bass_kernel_spmd(
        nc, in_maps, core_ids=list(range(NCORES)), trace=PROFILE
    )
    if PROFILE:
        LAST_EXEC_NS = res.exec_time_ns
        LAST_TRACE = (
            res.instructions_and_trace[1] if res.instructions_and_trace else None
        )
    acc = np.asarray(x, dtype=np.float32).copy()
    for c in range(NCORES):
        acc += res.results[c]["partial"][:T, :H].astype(np.float32)
    return acc.astype(BF16)


# revision 8
# speedup vs baseline: 128.8109x; 1.0022x over previous
"""MoE MLP block (RMSNorm + top-4-of-32 router + 32-expert SwiGLU MLP +
weighted combine + residual) on 8 Trainium2 NeuronCores.

Expert-parallel with count-aware slot packing: the host routes tokens
(reusing the RMSNorm it computes for the router logits as the device
token table), bin-packs the 32 experts into 8 cores x a shared slot
pattern (e.g. [2,2,1,1] m-tiles of 128 rows) so padded matmul work
tracks the real expert load, and ships layout-prepped weights.

Per core, per expert slot, the device:
  - dma_gather(transpose=True): gathers the slot's token rows from the
    normed token table AND transposes them into [128, stripe, M] lhsT
    layout in one SWDGE DMA (a ones-column at position 2880 folds b1).
  - W1 matmuls (full 5.65MB weight chunks; double-buffered), swiglu,
    PE-transpose a -> aT, W2 matmuls (+b2 via ones row), combine-weight
    row scaling.
  - dma_scatter_add: accumulates the weighted rows straight into the
    per-core partial output in DRAM (padding rows carry zeros into a
    dummy row; the runtime pre-zeros ExternalOutput buffers).

Chunk-level software pipelining keeps the tensor engine in long bursts
(p-state) while the weight stream saturates the DMA engines: slot s-1's
W2 chunk matmuls interleave between slot s's W1 chunk loads (W2
lagged one chunk so W1 deliveries lead tensor-engine demand); gathers
prefetch one slot ahead; the opening W1 load issues from the SWDGE
queue so the first gathers win the DMA-pool FIFO. The host sums the 8
partials and adds the residual.
"""

import functools
import sys

import numpy as np

sys.path.insert(0, "/opt/trn_rl_repo")

import ml_dtypes  # noqa: E402

import concourse.bass as bass  # noqa: E402
import concourse.tile as tile  # noqa: E402
from concourse import bacc, mybir  # noqa: E402
from concourse.bass_utils import run_bass_kernel_spmd  # noqa: E402
from concourse.masks import make_identity  # noqa: E402

BF16 = ml_dtypes.bfloat16

PROFILE = False      # set by test.py; harness leaves it False
LAST_EXEC_NS = None  # slowest-core HW exec time when PROFILE
LAST_TRACE = None    # perfetto trace path when PROFILE
LAST_PAT = None      # slot pattern of the last kernel() call

T, H, I, E, K = 1024, 2880, 2880, 32, 4
LIMIT, ALPHA, EPS, CAP = 7.0, 1.702, 1e-5, 384
NCORES = 8
NSLOT = E // NCORES   # expert slots per core
HP = 2944             # padded row length = 23 * 128 (H + bias + zeros)
NS = 23               # contraction stripes of 128 over HP
CN = 480              # free-dim chunk width (PSUM bank holds 512 fp32)
NCH = I // CN         # 6 chunks over the glu/lin halves and over H

AF = mybir.ActivationFunctionType
ALU = mybir.AluOpType


# ---------------------------------------------------------------------------
# Device program
# ---------------------------------------------------------------------------
@functools.lru_cache(maxsize=4)
def _build_program(pat: tuple):
    """pat: per-slot m-tile counts, e.g. (2, 2, 1, 1)."""
    dt = mybir.dt
    nc = bacc.Bacc(
        "TRN2", target_bir_lowering=False, debug=False, num_devices=NCORES
    )
    MTOT = sum(pat)  # total m-tiles per core
    t_d = nc.dram_tensor("tnorm", [T, HP], dt.bfloat16, kind="ExternalInput").ap()
    w1_d = nc.dram_tensor(
        "w1p", [NSLOT, HP, 2 * I], dt.bfloat16, kind="ExternalInput"
    ).ap()
    w2_d = nc.dram_tensor(
        "w2p", [NSLOT, HP, H], dt.bfloat16, kind="ExternalInput"
    ).ap()
    gidx_d = nc.dram_tensor(
        "gidx", [MTOT, 128, 8], dt.int16, kind="ExternalInput"
    ).ap()
    sidx_d = nc.dram_tensor(
        "sidx", [MTOT, 128, 8], dt.int16, kind="ExternalInput"
    ).ap()
    coef_d = nc.dram_tensor(
        "coef", [MTOT, 128], dt.float32, kind="ExternalInput"
    ).ap()
    # partial is pre-zeroed by the runtime (ExternalOutput semantics)
    part_d = nc.dram_tensor(
        "partial", [T + 128, HP], dt.bfloat16, kind="ExternalOutput"
    ).ap()

    mt0 = [0]
    for s in range(NSLOT):
        mt0.append(mt0[-1] + pat[s])

    with tile.TileContext(nc) as tc:
        with (
            tc.tile_pool(name="const", bufs=1) as const,
            tc.tile_pool(name="xeT", bufs=2) as xetp,
            tc.tile_pool(name="aT", bufs=1) as atp,
            tc.tile_pool(name="asb", bufs=1) as asbp,
            tc.tile_pool(name="osb", bufs=1) as osbp,
            tc.tile_pool(name="w1s", bufs=2) as w1p,
            tc.tile_pool(name="w2s", bufs=2) as w2p,
            tc.tile_pool(name="sw", bufs=2) as swp,
            tc.tile_pool(name="small", bufs=4) as smp,
            tc.tile_pool(name="ps1", bufs=1, space="PSUM") as psp,
            tc.tile_pool(name="ps2", bufs=2, space="PSUM") as pst,
        ):
            identity = const.tile([128, 128], dt.bfloat16)
            make_identity(nc, identity[:])

            # live per-slot state
            xeT = {}   # slot -> [m] tiles [128, NS*128]
            a_sb = {}  # slot -> [m] tiles [128, I]
            aT = {}    # slot -> [m] tiles [128, NS*128]
            o_sb = {}  # slot -> [m] tiles [128, HP]
            cf = {}    # slot -> tile [128, MT]

            def gather(s):
                MT = pat[s]
                xeT[s] = [
                    xetp.tile([128, NS * 128], dt.bfloat16,
                              tag=f"xeT{m}", name=f"xeT{s}_{m}")
                    for m in range(MT)
                ]
                for m in range(MT):
                    gi = smp.tile([128, 8], dt.int16, tag=f"gi{m}")
                    nc.sync.dma_start(out=gi[:], in_=gidx_d[mt0[s] + m, :, :])
                    nc.gpsimd.dma_gather(
                        out_ap=xeT[s][m][:].rearrange("p (s j) -> p s j", s=NS),
                        in_ap=t_d[:, :],
                        idxs_ap=gi[:],
                        num_idxs=128,
                        num_idxs_reg=128,
                        elem_size=HP,
                        transpose=True,
                    )

            def w1_load(s, n, eng=None):
                w1s = w1p.tile([128, NS * 2 * CN], dt.bfloat16, tag="w1s")
                (eng or nc.sync).dma_start(
                    out=w1s[:].rearrange("p (s c) -> p s c", s=NS),
                    in_=w1_d[
                        s, :, n * 2 * CN : (n + 1) * 2 * CN
                    ].rearrange("(s p) c -> p s c", p=128),
                )
                return w1s

            def w1_matmuls(s, n, w1s):
                MT = pat[s]
                pg = [
                    psp.tile([128, CN], dt.float32, space="PSUM",
                             tag=f"A{m}", name=f"pg{s}_{n}_{m}")
                    for m in range(MT)
                ]
                pl = [
                    psp.tile([128, CN], dt.float32, space="PSUM",
                             tag=f"B{m}", name=f"pl{s}_{n}_{m}")
                    for m in range(MT)
                ]
                for st in range(NS):
                    c0 = st * 2 * CN
                    for m in range(MT):
                        nc.tensor.matmul(
                            out=pg[m][:],
                            lhsT=xeT[s][m][:, st * 128 : (st + 1) * 128],
                            rhs=w1s[:, c0 : c0 + CN],
                            start=(st == 0),
                            stop=(st == NS - 1),
                        )
                        nc.tensor.matmul(
                            out=pl[m][:],
                            lhsT=xeT[s][m][:, st * 128 : (st + 1) * 128],
                            rhs=w1s[:, c0 + CN : c0 + 2 * CN],
                            start=(st == 0),
                            stop=(st == NS - 1),
                        )
                return pg, pl

            def swiglu(s, n, pg, pl):
                MT = pat[s]
                if n == 0:
                    a_sb[s] = [
                        asbp.tile([128, I], dt.bfloat16, tag=f"a{m}",
                                  name=f"a{s}_{m}")
                        for m in range(MT)
                    ]
                for m in range(MT):
                    xg = swp.tile([128, CN], dt.float32, tag="xg")
                    nc.vector.tensor_scalar_min(
                        out=xg[:], in0=pg[m][:], scalar1=LIMIT
                    )
                    sg = swp.tile([128, CN], dt.float32, tag="sg")
                    nc.scalar.activation(
                        out=sg[:], in_=xg[:], func=AF.Sigmoid, scale=ALPHA
                    )
                    nc.vector.tensor_mul(out=xg[:], in0=xg[:], in1=sg[:])
                    xl = swp.tile([128, CN], dt.float32, tag="xl")
                    nc.vector.tensor_scalar(
                        out=xl[:],
                        in0=pl[m][:],
                        scalar1=LIMIT,
                        scalar2=-LIMIT,
                        op0=ALU.min,
                        op1=ALU.max,
                    )
                    nc.vector.tensor_scalar_add(out=xl[:], in0=xl[:], scalar1=1.0)
                    nc.vector.tensor_mul(
                        out=a_sb[s][m][:, n * CN : (n + 1) * CN],
                        in0=xg[:],
                        in1=xl[:],
                    )

            def transposes(s):
                MT = pat[s]
                aT[s] = [
                    atp.tile([128, NS * 128], dt.bfloat16, tag=f"aT{m}",
                             name=f"aT{s}_{m}")
                    for m in range(MT)
                ]
                for m in range(MT):
                    for st in range(NS):
                        hsz = 128 if st < 22 else 64
                        tp = pst.tile([hsz, 128], dt.bfloat16, space="PSUM",
                                      tag="T")
                        nc.tensor.transpose(
                            out=tp[:],
                            in_=a_sb[s][m][:, st * 128 : st * 128 + hsz],
                            identity=identity[:],
                        )
                        nc.scalar.copy(
                            out=aT[s][m][0:hsz, st * 128 : (st + 1) * 128],
                            in_=tp[:],
                        )
                        if st == 22:
                            nc.vector.memset(
                                aT[s][m][64:128, st * 128 : (st + 1) * 128], 0.0
                            )
                            nc.vector.memset(
                                aT[s][m][64:65, st * 128 : (st + 1) * 128], 1.0
                            )
                # combine coefficients for this slot
                cf[s] = smp.tile([128, MT], dt.float32, tag="cf",
                                 name=f"cf{s}")
                for m in range(MT):
                    nc.sync.dma_start(
                        out=cf[s][:, m : m + 1],
                        in_=coef_d[mt0[s] + m, :].rearrange("(a b) -> a b", b=1),
                    )

            def w2_load(s, n):
                w2s = w2p.tile([128, NS * CN], dt.bfloat16, tag="w2s")
                nc.sync.dma_start(
                    out=w2s[:].rearrange("p (s c) -> p s c", s=NS),
                    in_=w2_d[s, :, n * CN : (n + 1) * CN].rearrange(
                        "(s p) c -> p s c", p=128
                    ),
                )
                return w2s

            def w2_matmuls(s, n, w2s):
                MT = pat[s]
                if n == 0:
                    o_sb[s] = [
                        osbp.tile([128, HP], dt.bfloat16, tag=f"o{m}",
                                  name=f"o{s}_{m}")
                        for m in range(MT)
                    ]
                po = [
                    psp.tile([128, CN], dt.float32, space="PSUM",
                             tag=f"C{m}", name=f"po{s}_{n}_{m}")
                    for m in range(MT)
                ]
                for st in range(NS):
                    c0 = st * CN
                    for m in range(MT):
                        nc.tensor.matmul(
                            out=po[m][:],
                            lhsT=aT[s][m][:, st * 128 : (st + 1) * 128],
                            rhs=w2s[:, c0 : c0 + CN],
                            start=(st == 0),
                            stop=(st == NS - 1),
                        )
                for m in range(MT):
                    nc.vector.tensor_scalar_mul(
                        out=o_sb[s][m][:, n * CN : (n + 1) * CN],
                        in0=po[m][:],
                        scalar1=cf[s][:, m : m + 1],
                    )

            def scatter(s):
                MT = pat[s]
                for m in range(MT):
                    nc.vector.memset(o_sb[s][m][:, H:HP], 0.0)
                    si = smp.tile([128, 8], dt.int16, tag=f"si{m}")
                    nc.sync.dma_start(out=si[:], in_=sidx_d[mt0[s] + m, :, :])
                    nc.gpsimd.dma_scatter_add(
                        part_d[:, :],
                        o_sb[s][m][:].rearrange("p (a c) -> p a c", a=1),
                        si[:],
                        128,
                        128,
                        HP,
                    )

            # ---- pipeline ----
            # First two slots' gathers go first; the opening W1 load is
            # issued from the Pool (SWDGE) queue so its descriptors are
            # generated strictly after the gathers' — the gathers' small
            # transfers win the DMA-pool FIFO and the PE cold start shrinks.
            gather(0)
            gather(1)
            w1s = w1_load(0, 0, eng=nc.gpsimd)
            for n in range(NCH):
                pg, pl = w1_matmuls(0, n, w1s)
                if n + 1 < NCH:
                    w1s = w1_load(0, n + 1)
                swiglu(0, n, pg, pl)
            for s in range(1, NSLOT):
                transposes(s - 1)
                if s + 1 < NSLOT:
                    gather(s + 1)
                # W2 stream lags W1 by one chunk: the DMA queue front-loads
                # W1 so its last chunk lands one W2-transfer earlier than
                # the tensor engine needs it at the step boundary
                w1s = w1_load(s, 0)
                pg, pl = w1_matmuls(s, 0, w1s)
                swiglu(s, 0, pg, pl)
                for n in range(1, NCH):
                    w1s = w1_load(s, n)
                    pg, pl = w1_matmuls(s, n, w1s)
                    w2s = w2_load(s - 1, n - 1)
                    w2_matmuls(s - 1, n - 1, w2s)
                    swiglu(s, n, pg, pl)
                w2s = w2_load(s - 1, NCH - 1)
                w2_matmuls(s - 1, NCH - 1, w2s)
                scatter(s - 1)
            # tail: prefetch the first W2 chunk ahead of the transposes
            w2s_t = w2_load(NSLOT - 1, 0)
            transposes(NSLOT - 1)
            w2_matmuls(NSLOT - 1, 0, w2s_t)
            for n in range(1, NCH):
                w2s = w2_load(NSLOT - 1, n)
                w2_matmuls(NSLOT - 1, n, w2s)
            scatter(NSLOT - 1)

    nc.compile()
    return nc


# ---------------------------------------------------------------------------
# Host-side routing (mirrors reference semantics; O(T*E) work only)
# ---------------------------------------------------------------------------
def _route(x, norm_scale, gate_w, gate_b):
    xf = np.asarray(x, dtype=np.float32)
    ms = np.mean(xf * xf, axis=-1, keepdims=True)
    t32 = xf / np.sqrt(ms + EPS)
    t32 = t32 * np.asarray(norm_scale, dtype=np.float32)
    tb = t32.astype(BF16)
    tbf = tb.astype(np.float32)
    g = (tbf @ np.asarray(gate_w, dtype=np.float32)).astype(BF16).astype(np.float32)
    g = (g + np.asarray(gate_b, dtype=np.float32)).astype(BF16).astype(np.float32)
    # top-k with lowest-index tie-break (matches jax.lax.top_k)
    top_idx = np.argsort(-g, axis=-1, kind="stable")[:, :K].astype(np.int32)
    top_vals = np.take_along_axis(g, top_idx, axis=-1)
    ex = np.exp(top_vals - top_vals.max(axis=-1, keepdims=True))
    top_w = (ex / ex.sum(axis=-1, keepdims=True)).astype(BF16).astype(np.float32)

    N = T * K
    e_flat = top_idx.reshape(N)
    w_flat = top_w.reshape(N)
    tok_flat = np.repeat(np.arange(T, dtype=np.int32), K)
    order = np.argsort(e_flat, kind="stable")
    se, sw, stok = e_flat[order], w_flat[order], tok_flat[order]
    counts = np.bincount(se, minlength=E)
    starts = np.cumsum(counts) - counts
    pos = np.arange(N, dtype=np.int64) - starts[se]
    valid = pos < CAP
    return tb, se, sw, stok, pos, valid, counts


def _wrap16(idx):
    """[M] int -> [128, M//16] int16: idx i at [i % 16, i // 16], replicated
    into each of the 8 q7-core 16-partition groups."""
    m = idx.shape[0]
    w = np.zeros((128, m // 16), np.int16)
    w[:16, :] = idx.astype(np.int16).reshape(m // 16, 16).T
    w[:] = np.tile(w[:16], (8, 1))
    return w


def kernel(x, norm_scale, gate_w, gate_b, W1, b1, W2, b2):
    x = np.asarray(x)
    tb, se, sw, stok, pos, valid, counts = _route(x, norm_scale, gate_w, gate_b)

    # ---- bin-pack experts into 8 cores x slot pattern -------------------
    tiles = np.maximum(1, np.ceil(np.minimum(counts, CAP) / 128).astype(int))
    order_e = np.argsort(-tiles, kind="stable")  # big experts first
    core_exp = [[] for _ in range(NCORES)]
    core_load = np.zeros(NCORES, int)
    for e in order_e:
        c = np.argmin(
            core_load + (np.array([len(ce) for ce in core_exp]) >= NSLOT) * 1000
        )
        core_exp[int(c)].append(int(e))
        core_load[int(c)] += tiles[e]
    pat = np.zeros(NSLOT, int)
    for c in range(NCORES):
        ts = sorted([tiles[e] for e in core_exp[c]], reverse=True)
        for s in range(NSLOT):
            pat[s] = max(pat[s], ts[s])
        core_exp[c] = sorted(core_exp[c], key=lambda e: -tiles[e])
    pat = tuple(int(v) for v in pat)
    MTOT = sum(pat)

    # ---- dispatch metadata ---------------------------------------------
    gidx = np.zeros((NCORES, MTOT, 128), np.int32)
    sidx = np.full((NCORES, MTOT, 128), T, np.int32)  # dummy row = T
    coef = np.zeros((NCORES, MTOT, 128), np.float32)
    ok = valid
    for c in range(NCORES):
        m0 = 0
        for s_i, e in enumerate(core_exp[c]):
            sel = ok & (se == e) & (pos < pat[s_i] * 128)
            tok_e = stok[sel]
            w_e = sw[sel]
            ncnt = tok_e.shape[0]
            gidx[c, m0 : m0 + pat[s_i]].reshape(-1)[:ncnt] = tok_e
            sidx[c, m0 : m0 + pat[s_i]].reshape(-1)[:ncnt] = tok_e
            coef[c, m0 : m0 + pat[s_i]].reshape(-1)[:ncnt] = w_e
            m0 += pat[s_i]

    # ---- token table: normed tokens + ones column + zero pad ------------
    t_pad = np.zeros((T, HP), BF16)
    t_pad[:, :H] = tb
    t_pad[:, H] = BF16(1.0)

    # ---- per-core weight prep ------------------------------------------
    j = np.arange(CN)
    col_perm = np.concatenate(
        [np.concatenate([2 * (n * CN + j), 2 * (n * CN + j) + 1]) for n in range(NCH)]
    )
    W1 = np.asarray(W1)
    b1 = np.asarray(b1)
    W2 = np.asarray(W2)
    b2 = np.asarray(b2)

    global LAST_PAT
    LAST_PAT = pat
    nc = _build_program(pat)
    in_maps = []
    for c in range(NCORES):
        es = core_exp[c]
        w1p = np.zeros((NSLOT, HP, 2 * I), BF16)
        w1p[:, :H, :] = W1[es][:, :, col_perm]
        w1p[:, H, :] = b1[es][:, col_perm]
        w2p = np.zeros((NSLOT, HP, H), BF16)
        w2p[:, :H, :] = W2[es]
        w2p[:, H, :] = b2[es]
        in_maps.append(
            {
                "tnorm": t_pad,
                "w1p": w1p,
                "w2p": w2p,
                "gidx": np.stack([_wrap16(gidx[c, m]) for m in range(MTOT)]),
                "sidx": np.stack([_wrap16(sidx[c, m]) for m in range(MTOT)]),
                "coef": coef[c],
            }
        )

    global LAST_EXEC_NS, LAST_TRACE
    res = run_bass_kernel_spmd(
        nc, in_maps, core_ids=list(range(NCORES)), trace=PROFILE
    )
    if PROFILE:
        LAST_EXEC_NS = res.exec_time_ns
        LAST_TRACE = (
            res.instructions_and_trace[1] if res.instructions_and_trace else None
        )
    acc = np.asarray(x, dtype=np.float32).copy()
    for c in range(NCORES):
        acc += res.results[c]["partial"][:T, :H].astype(np.float32)
    return acc.astype(BF16)
